# revision 32
# baseline (speedup 1.0000x reference)
"""Trainium2 Bass kernel for nn_AdvancedTransformerBlock_15006615733156.

Pre-norm transformer block: LN1 -> QKV -> sliding-window causal attention with
ALiBi (window 256) -> proj residual -> LN2 -> FFN (exact GELU) residual.
B=2, N=2048, D=1024, H=16, HD=64.

Sharding: 8 cores = batch(2) x sequence(4 chunks of 512 rows). The 256-wide
sliding window means each core only needs a 256-row halo of x before its
chunk — no collectives. Chunk-0 cores get a zeroed halo plus a `valid` mask
that zeroes halo V' rows (kills both numerator and softmax denominator).

On-chip layout: scores are computed transposed (S_t[kj, qi]) so the
probability tile is directly consumable as matmul lhsT for PV; the softmax
denominator comes from an appended ones-column in V'. All big matmuls run in
float32r (full PE rate, ~1.4e-4 rounding). LN stats use ACT accum_out.
"""
import sys, math, os
sys.path.insert(0, '/opt/trn_rl_repo')
import numpy as np

B, N, D, H, HD, WIN = 2, 2048, 1024, 16, 64, 256
CHUNK, HALO, ROWS = 512, 256, 768
NEG = -1e30
LN_EPS = 1e-5
NCORES = 8

_cache = {}


def _alibi_slopes(n):
    closest = 2 ** math.floor(math.log2(n))
    base = 2.0 ** (-(2.0 ** (-(math.log2(closest) - 3))))
    return np.power(base, np.arange(1, closest + 1)).astype(np.float32)


def _build_amask():
    """Additive pre-softmax bias, [H, 128, 384] bf16.

    Softmax over keys j is invariant to any per-query-column constant, so the
    reference's "+1 inside window" and the -slope*i part of the ALiBi term
    drop out; what remains is slope*(j - i) <= 0 inside the band, -1e30
    outside. Values near each column's max are near zero, so bf16's relative
    rounding cannot disturb the softmax weights meaningfully.
    """
    import ml_dtypes
    sl = _alibi_slopes(H)
    j = np.arange(128)[:, None]
    i = np.arange(384)[None, :]
    band = ((i - j) >= 0) & ((i - j) <= 255)
    out = np.where(band[None], sl[:, None, None] * (j - i)[None], NEG)
    return np.ascontiguousarray(out.astype(ml_dtypes.bfloat16))


def _kb_span(kb):
    qlo = max(0, kb * 128 - 256)
    qhi = min(512, kb * 128 + 128)
    return qlo, qhi, qlo - (kb * 128 - 256)


_KB_OFF = [0]
for _kb in range(6):
    _q0, _q1, _ = _kb_span(_kb)
    _KB_OFF.append(_KB_OFF[-1] + (_q1 - _q0))  # offsets into the 1536-wide S row


def _build_program(has_qk_bias, has_fc1_bias):
    import concourse.bass as bass
    import concourse.tile as tile
    from concourse import bacc, mybir
    from concourse.masks import make_identity

    F32, F32R = mybir.dt.float32, mybir.dt.float32r
    BF16 = mybir.dt.bfloat16
    # 16-bit P/V' runs PV at 1 cyc/row (fp32 pays 4x at N=65) and gets FWL on
    # the weight load; fp16's 10-bit mantissa keeps softmax-prob rounding at
    # ~5e-4 (bf16's 8-bit costs ~4x more accuracy).
    _pv = os.environ.get("K_PV_DT", "f16")
    PV_DT = {"f32": F32, "bf16": BF16, "f16": mybir.dt.float16}[_pv]
    AF = mybir.ActivationFunctionType
    ADD, MULT = mybir.AluOpType.add, mybir.AluOpType.mult

    nc = bacc.Bacc("TRN2", target_bir_lowering=False, debug=False,
                   num_devices=NCORES)

    xh_d = nc.dram_tensor("xh", [HALO, D], F32, kind="ExternalInput").ap()
    xl_d = nc.dram_tensor("xl", [CHUNK, D], F32, kind="ExternalInput").ap()
    wqkv_d = nc.dram_tensor("wqkv", [D, 3 * D], F32R, kind="ExternalInput").ap()
    wproj_d = nc.dram_tensor("wproj", [D, D], F32R, kind="ExternalInput").ap()
    wfc1_d = nc.dram_tensor("wfc1", [D, 4 * D], F32R, kind="ExternalInput").ap()
    wfc2_d = nc.dram_tensor("wfc2", [4 * D, D], F32R, kind="ExternalInput").ap()
    amask_d = nc.dram_tensor("amask", [H, 128, 384], BF16, kind="ExternalInput").ap()
    valid_d = nc.dram_tensor("valid", [6, 128], F32, kind="ExternalInput").ap()
    if has_qk_bias:
        qkb_d = nc.dram_tensor("qkbias", [2, 8, 128], F32, kind="ExternalInput").ap()
    if has_fc1_bias:
        b1_d = nc.dram_tensor("b1", [4 * D], F32, kind="ExternalInput").ap()
    y_d = nc.dram_tensor("y", [CHUNK, D], F32, kind="ExternalOutput").ap()

    def ln_block(tc, x_ap, out_ap, small, dump, eps):
        """LayerNorm (no affine) of [128, 1024]: out = (x - mu) * rstd.

        Stats split across engines in parallel: ACT computes E[x^2] via
        Square+accum while DVE reduces E[x]; var = E[x^2] - mu^2.
        """
        sq = small.tile([128, 1], F32, tag="sq", name="sq")
        nc.scalar.activation(dump[:], x_ap, AF.Square, accum_out=sq[:])
        sums = small.tile([128, 1], F32, tag="sums", name="sums")
        nc.vector.tensor_reduce(sums[:], x_ap, mybir.AxisListType.X,
                                mybir.AluOpType.add)
        negmu = small.tile([128, 1], F32, tag="negmu", name="negmu")
        nc.vector.tensor_scalar_mul(negmu[:], sums[:], -1.0 / D)
        m2 = small.tile([128, 1], F32, tag="m2", name="m2")
        nc.vector.tensor_tensor(m2[:], negmu[:], negmu[:], MULT)
        bvar = small.tile([128, 1], F32, tag="bvar", name="bvar")
        nc.vector.tensor_scalar(bvar[:], m2[:], -1.0, LN_EPS, MULT, ADD)
        st = small.tile([128, 1], F32, tag="st", name="st")
        nc.scalar.activation(st[:], sq[:], AF.Sqrt, bias=bvar[:], scale=1.0 / D)
        rstd = small.tile([128, 1], F32, tag="rstd", name="rstd")
        nc.vector.reciprocal(rstd[:], st[:])
        nmr = small.tile([128, 1], F32, tag="nmr", name="nmr")
        nc.vector.tensor_tensor(nmr[:], negmu[:], rstd[:], MULT)
        nc.vector.tensor_scalar(out_ap, x_ap, rstd[:], nmr[:], MULT, ADD)

    with tile.TileContext(nc) as tc:
        # Pool lifetimes form two LIFO stacks (SBUF left/right). Right holds
        # whole-kernel + B..C + F pools; left holds A..B, C..D, E..F1 chains.
        glob = tc.alloc_tile_pool(name="glob", bufs=1, side="right")
        small = tc.alloc_tile_pool(name="small", bufs=8, side="right")
        de = tc.alloc_tile_pool(name="de", bufs=1, side="right")  # x2 (D..end)

        ident = glob.tile([128, 128], F32, name="ident")
        make_identity(nc, ident[:])
        eps_t = glob.tile([128, 1], F32, name="eps_t")
        nc.vector.memset(eps_t[:], LN_EPS)
        dump = glob.tile([128, D], F32, name="dump")
        x2_sb = de.tile([128, 4, D], F32, name="x2_sb")
        amask_t = glob.tile([128, H, 384], BF16, name="amask_t")

        # ---------------- Phase A: LN1 + h^T ----------------
        xlp = tc.alloc_tile_pool(name="xlp", bufs=1, side="left")  # x local, A..D
        ab = tc.alloc_tile_pool(name="ab", bufs=1, side="left")
        hT = ab.tile([128, 8, ROWS], F32R, name="hT")

        pa = tc.alloc_tile_pool(name="pa", bufs=2, side="left")
        psa = tc.alloc_tile_pool(name="psa", bufs=2, space="PSUM")
        xh_sb = pa.tile([128, 2, D], F32, name="xh_sb", bufs=1)
        # halo first (LN block 0 needs it; HWDGE DMAs are FIFO), split so
        # block 0 lands early
        xh_r = xh_d.rearrange("(q p) d -> p q d", p=128)
        nc.sync.dma_start(xh_sb[:, 0], xh_r[:, 0])
        nc.sync.dma_start(xh_sb[:, 1], xh_r[:, 1])
        xl_sb = xlp.tile([128, 4, D], F32, name="xl_sb")
        xl_r = xl_d.rearrange("(q p) d -> p q d", p=128)
        for q in range(4):
            nc.sync.dma_start(xl_sb[:, q], xl_r[:, q])
        nc.scalar.dma_start(amask_t[:], amask_d.rearrange("h p c -> p h c"))
        for blk in range(6):
            x_ap = xh_sb[:, blk] if blk < 2 else xl_sb[:, blk - 2]
            h_pre = pa.tile([128, D], F32, tag="h_pre", name="h_pre")
            ln_block(tc, x_ap, h_pre[:], small, dump, eps_t)
            for kc in range(8):
                pst = psa.tile([128, 128], F32, tag="tr", name="ps_tr")
                nc.tensor.transpose(pst[:], h_pre[:, kc * 128:(kc + 1) * 128],
                                    ident[:])
                if kc % 2 == 0:
                    nc.vector.tensor_copy(hT[:, kc, blk * 128:(blk + 1) * 128], pst[:])
                else:
                    nc.scalar.copy(hT[:, kc, blk * 128:(blk + 1) * 128], pst[:])
        psa.release()
        pa.release()

        # ---------------- Phase B: QKV projections ----------------
        bc = tc.alloc_tile_pool(name="bc", bufs=1, side="right")
        QT = bc.tile([128, 8, CHUNK], F32R, name="QT")     # [hd-pair, pair, qi]
        KT = bc.tile([128, 8, ROWS], F32R, name="KT")
        Vp = bc.tile([128, 6, 16 * 65], PV_DT, name="Vp")    # per-head 65-col groups
        valid_t = bc.tile([128, 6], F32, name="valid_t")
        nc.scalar.dma_start(valid_t[:], valid_d.rearrange("k p -> p k"))
        if has_qk_bias:
            qkb_t = bc.tile([128, 2, 8], F32, name="qkb_t")
            nc.sync.dma_start(qkb_t[:], qkb_d.rearrange("t g p -> p t g"))

        wb = tc.alloc_tile_pool(name="wb", bufs=2, side="right")
        psb = tc.alloc_tile_pool(name="psb", bufs=2, space="PSUM")

        # Q: 4 groups of 2 head-pairs (cols 256 each)
        for g in range(4):
            wq = wb.tile([128, 8, 256], F32R, tag="wqk", name="wq")
            nc.sync.dma_start(
                wq[:], wqkv_d[:, g * 256:(g + 1) * 256]
                .rearrange("(ko p) n -> p ko n", p=128))
            for pp in range(2):
                p = g * 2 + pp
                psq = psb.tile([128, CHUNK], F32, tag="q", name="ps_q", bufs=2)
                for ko in range(8):
                    nc.tensor.matmul(psq[:], wq[:, ko, pp * 128:(pp + 1) * 128],
                                     hT[:, ko, HALO:ROWS],
                                     start=(ko == 0), stop=(ko == 7))
                if has_qk_bias:
                    nc.scalar.activation(QT[:, p], psq[:], AF.Identity,
                                         bias=qkb_t[:, 0, p:p + 1])
                else:
                    nc.scalar.copy(QT[:, p], psq[:])

        # K: 4 groups of 2 head-pairs, N=768
        for g in range(4):
            wk = wb.tile([128, 8, 256], F32R, tag="wqk", name="wk")
            nc.sync.dma_start(
                wk[:], wqkv_d[:, D + g * 256:D + (g + 1) * 256]
                .rearrange("(ko p) n -> p ko n", p=128))
            for pp in range(2):
                p = g * 2 + pp
                psk = psb.tile([128, ROWS], F32, tag="k", name="ps_k", bufs=2)
                for n0, n1 in ((0, 512), (512, 768)):
                    for ko in range(8):
                        nc.tensor.matmul(psk[:, n0:n1],
                                         wk[:, ko, pp * 128:(pp + 1) * 128],
                                         hT[:, ko, n0:n1],
                                         start=(ko == 0), stop=(ko == 7))
                if has_qk_bias:
                    nc.scalar.activation(KT[:, p], psk[:], AF.Identity,
                                         bias=qkb_t[:, 1, p:p + 1])
                else:
                    nc.scalar.copy(KT[:, p], psk[:])

        # V: natural layout [rows, feats], assembled into V' with ones col
        wv = wb.tile([128, 8, D], F32R, tag="wv", name="wv", bufs=1)
        nc.sync.dma_start(wv[:],
                          wqkv_d[:, 2 * D:3 * D].rearrange("(ko p) n -> p ko n", p=128))
        for rb in range(6):
            vp_rb = Vp[:, rb].rearrange("p (h c) -> p h c", c=65)
            for nh in range(2):
                psv = psb.tile([128, 512], F32, tag="v", name="ps_v", bufs=2)
                for ko in range(8):
                    nc.tensor.matmul(psv[:],
                                     hT[:, ko, rb * 128:(rb + 1) * 128],
                                     wv[:, ko, nh * 512:(nh + 1) * 512],
                                     start=(ko == 0), stop=(ko == 7))
                # heads nh*8 .. nh*8+8 of this row-block
                nc.vector.tensor_copy(
                    vp_rb[:, nh * 8:(nh + 1) * 8, 0:64],
                    psv[:].rearrange("p (h c) -> p h c", c=64))
            nc.vector.memset(vp_rb[:, :, 64:65], 1.0)
            nc.vector.tensor_scalar_mul(Vp[:, rb], Vp[:, rb], valid_t[:, rb:rb + 1])

        psb.release()
        wb.release()
        ab.release()  # frees hT

        # ---------------- Phase C: attention ----------------
        cd = tc.alloc_tile_pool(name="cd", bufs=1, side="left")
        O_sb = cd.tile([128, 4, D], F32, name="O_sb")

        sp = tc.alloc_tile_pool(name="sp", bufs=2, side="right")
        ddw = tc.alloc_tile_pool(name="ddw", bufs=1, side="left")
        psc = tc.alloc_tile_pool(name="psc", bufs=4, space="PSUM")
        pso = tc.alloc_tile_pool(name="pso", bufs=4, space="PSUM")
        wpj = None
        for hp in range(8):
            if hp == 4:
                # prefetch proj weights while attention still runs
                wpj = ddw.tile([128, 8, D], F32R, name="wpj")
                nc.sync.dma_start(wpj[:],
                                  wproj_d.rearrange("(ko p) n -> p ko n", p=128))
            S_pr = sp.tile([128, 2, 1536], F32, tag="S", name="S_pr")
            P_pr = sp.tile([128, 2, 1536], PV_DT, tag="P", name="P_pr")
            for kb in range(6):
                qlo, qhi, il = _kb_span(kb)
                w = qhi - qlo
                # widen 128-wide spans to 256 so fp32r streams at 1 cyc/row;
                # the extra columns stay in PSUM and are never read.
                mlo = min(qlo, 256) if w >= 256 else (0 if kb == 0 else 256)
                mhi = mlo + max(w, 256)
                voff = qlo - mlo
                # [128, 2, 512]: each head-half starts on a PSUM bank boundary
                pss = psc.tile([128, 2, 512], F32, tag="s", name="ps_s", bufs=2)
                for hh in range(2):
                    pb = hh * 64
                    nc.tensor.matmul(pss[:, hh, :mhi - mlo],
                                     KT[pb:pb + 64, hp, kb * 128:(kb + 1) * 128],
                                     QT[pb:pb + 64, hp, mlo:mhi],
                                     start=True, stop=True)
                nc.vector.tensor_tensor(
                    S_pr[:, :, _KB_OFF[kb]:_KB_OFF[kb] + w],
                    pss[:, :, voff:voff + w],
                    amask_t[:, 2 * hp:2 * hp + 2, il:il + w], ADD)
            nc.scalar.activation(P_pr[:], S_pr[:], AF.Exp)
            for hh in range(2):
                h_i = hp * 2 + hh
                for qb in range(4):
                    po = pso.tile([128, 65], F32, tag="o", name="ps_o")
                    for t in range(3):
                        kb = qb + t
                        qlo, _, _ = _kb_span(kb)
                        pcol = _KB_OFF[kb] + qb * 128 - qlo
                        nc.tensor.matmul(po[:], P_pr[:, hh, pcol:pcol + 128],
                                         Vp[:, kb, h_i * 65:(h_i + 1) * 65],
                                         start=(t == 0), stop=(t == 2))
                    rec = small.tile([128, 1], F32, tag="rec", name="rec")
                    nc.vector.reciprocal(rec[:], po[:, 64:65])
                    if qb % 2 == 0:
                        # balance the divide work across ACT and DVE
                        nc.scalar.activation(
                            O_sb[:, qb, h_i * 64:(h_i + 1) * 64], po[:, 0:64],
                            AF.Copy, scale=rec[:])
                    else:
                        nc.vector.tensor_scalar_mul(
                            O_sb[:, qb, h_i * 64:(h_i + 1) * 64], po[:, 0:64],
                            rec[:])
        pso.release()
        psc.release()
        sp.release()
        bc.release()  # frees QT/KT/Vp

        # ---------------- Phase D: O^T + proj + residual ----------------
        dd = tc.alloc_tile_pool(name="dd", bufs=1, side="left")
        OT = dd.tile([128, 8, CHUNK], F32R, name="OT")
        psd = tc.alloc_tile_pool(name="psd", bufs=2, space="PSUM")
        for qb in range(4):
            for fc in range(8):
                pst = psd.tile([128, 128], F32, tag="tr", name="ps_tr2")
                nc.tensor.transpose(pst[:], O_sb[:, qb, fc * 128:(fc + 1) * 128],
                                    ident[:])
                if fc % 2 == 0:
                    nc.vector.tensor_copy(OT[:, fc, qb * 128:(qb + 1) * 128], pst[:])
                else:
                    nc.scalar.copy(OT[:, fc, qb * 128:(qb + 1) * 128], pst[:])
        for qc in range(4):
            for nh in range(2):
                psp = psd.tile([128, 512], F32, tag="p", name="ps_p")
                for fc in range(8):
                    nc.tensor.matmul(psp[:], OT[:, fc, qc * 128:(qc + 1) * 128],
                                     wpj[:, fc, nh * 512:(nh + 1) * 512],
                                     start=(fc == 0), stop=(fc == 7))
                nc.vector.tensor_tensor(x2_sb[:, qc, nh * 512:(nh + 1) * 512],
                                        psp[:], xl_sb[:, qc, nh * 512:(nh + 1) * 512],
                                        ADD)
        psd.release()
        dd.release()
        ddw.release()
        cd.release()
        xlp.release()

        # ---------------- Phase F pools (right side, before E on left) ------
        ff = tc.alloc_tile_pool(name="ff", bufs=1, side="right")
        ffT = ff.tile([128, 32, CHUNK], F32R, name="ffT")
        y_sb = ff.tile([128, 4, D], F32, name="y_sb")
        if has_fc1_bias:
            b1_t = ff.tile([128, 32], F32, name="b1_t")
            nc.sync.dma_start(b1_t[:], b1_d.rearrange("(fo p) -> p fo", p=128))
        wf = tc.alloc_tile_pool(name="wf", bufs=2, side="right")

        # ---------------- Phase E: LN2 + h2^T ----------------
        ef = tc.alloc_tile_pool(name="ef", bufs=1, side="left")
        h2T = ef.tile([128, 8, CHUNK], F32R, name="h2T")
        pe_ = tc.alloc_tile_pool(name="pe", bufs=2, side="left")
        pse = tc.alloc_tile_pool(name="pse", bufs=2, space="PSUM")
        for qc in range(4):
            h2_pre = pe_.tile([128, D], F32, tag="h2_pre", name="h2_pre")
            ln_block(tc, x2_sb[:, qc], h2_pre[:], small, dump, eps_t)
            for kc in range(8):
                pst = pse.tile([128, 128], F32, tag="tr", name="ps_tr3")
                nc.tensor.transpose(pst[:], h2_pre[:, kc * 128:(kc + 1) * 128],
                                    ident[:])
                if kc % 2 == 0:
                    nc.vector.tensor_copy(h2T[:, kc, qc * 128:(qc + 1) * 128], pst[:])
                else:
                    nc.scalar.copy(h2T[:, kc, qc * 128:(qc + 1) * 128], pst[:])
        pse.release()
        pe_.release()

        # ---------------- Phase F1: fc1 + GELU ----------------
        psf = tc.alloc_tile_pool(name="psf", bufs=2, space="PSUM")
        for g in range(16):
            w1 = wf.tile([128, 8, 256], F32R, tag="w1", name="w1", bufs=3)
            nc.sync.dma_start(
                w1[:], wfc1_d[:, g * 256:(g + 1) * 256]
                .rearrange("(ko p) n -> p ko n", p=128))
            for f4 in range(2):
                ffc = g * 2 + f4
                psq = psf.tile([128, 512], F32, tag="f", name="ps_f")
                for ko in range(8):
                    nc.tensor.matmul(psq[:], w1[:, ko, f4 * 128:(f4 + 1) * 128],
                                     h2T[:, ko, :], start=(ko == 0), stop=(ko == 7))
                if has_fc1_bias:
                    nc.scalar.activation(ffT[:, ffc, :], psq[:], AF.Gelu,
                                         bias=b1_t[:, ffc:ffc + 1])
                else:
                    nc.scalar.activation(ffT[:, ffc, :], psq[:], AF.Gelu)
        psf.release()
        ef.release()

        # ---------------- Phase F2: fc2 + residual + store ----------------
        psy = tc.alloc_tile_pool(name="psy", bufs=1, space="PSUM")
        ys = [psy.tile([128, 512], F32, name=f"ps_y{i}") for i in range(8)]
        for ffc in range(32):
            w2 = wf.tile([128, D], F32R, tag="w2", name="w2", bufs=3)
            nc.sync.dma_start(w2[:], wfc2_d[ffc * 128:(ffc + 1) * 128, :])
            for qc in range(4):
                for nh in range(2):
                    nc.tensor.matmul(ys[qc * 2 + nh][:],
                                     ffT[:, ffc, qc * 128:(qc + 1) * 128],
                                     w2[:, nh * 512:(nh + 1) * 512],
                                     start=(ffc == 0), stop=(ffc == 31))
        y_dr = y_d.rearrange("(q p) d -> p q d", p=128)
        for qc in range(4):
            for nh in range(2):
                nc.vector.tensor_tensor(y_sb[:, qc, nh * 512:(nh + 1) * 512],
                                        ys[qc * 2 + nh][:],
                                        x2_sb[:, qc, nh * 512:(nh + 1) * 512], ADD)
            nc.scalar.dma_start(y_dr[:, qc], y_sb[:, qc])
        psy.release()
        wf.release()
        ff.release()
        de.release()
        small.release()
        glob.release()

    nc.compile()
    return nc


def kernel(x, qkv_w, qkv_b, proj_w, proj_b, ln1_g, ln1_b, ln2_g, ln2_b,
           fc1_w, fc1_b, fc2_w, fc2_b):
    from concourse.bass_utils import run_bass_kernel_spmd

    x = np.ascontiguousarray(np.asarray(x, dtype=np.float32))
    f32 = lambda a: np.asarray(a, dtype=np.float32)
    qkv_w, qkv_b = f32(qkv_w), f32(qkv_b)
    proj_w, proj_b = f32(proj_w), f32(proj_b)
    fc1_w, fc1_b = f32(fc1_w), f32(fc1_b)
    fc2_w, fc2_b = f32(fc2_w), f32(fc2_b)
    ln1_g, ln1_b = f32(ln1_g), f32(ln1_b)
    ln2_g, ln2_b = f32(ln2_g), f32(ln2_b)

    # Host-side folding: LN affine into the following weight/bias; HD^-0.5 into Wk.
    scale = HD ** -0.5
    wqkv = ln1_g[:, None] * qkv_w
    bqkv = qkv_b + ln1_b @ qkv_w
    wqkv = np.ascontiguousarray(wqkv)
    wqkv[:, D:2 * D] *= scale
    bqkv = bqkv.copy()
    bqkv[D:2 * D] *= scale
    wfc1 = np.ascontiguousarray(ln2_g[:, None] * fc1_w)
    bfc1 = fc1_b + ln2_b @ fc1_w

    if np.any(bqkv[2 * D:]) or np.any(proj_b) or np.any(fc2_b):
        raise NotImplementedError("nonzero v/proj/fc2 bias not supported")

    has_qk_bias = bool(np.any(bqkv[:2 * D]))
    has_fc1_bias = bool(np.any(bfc1))
    key = (has_qk_bias, has_fc1_bias)
    if key not in _cache:
        _cache[key] = _build_program(*key)
    nc = _cache[key]

    amask = _build_amask()
    in_maps = []
    for c in range(NCORES):
        b, ck = c // 4, c % 4
        g0 = ck * CHUNK
        xl = np.ascontiguousarray(x[b, g0:g0 + CHUNK])
        if ck > 0:
            xhalo = np.ascontiguousarray(x[b, g0 - HALO:g0])
        else:
            xhalo = np.zeros((HALO, D), np.float32)
        valid = np.ones((6, 128), np.float32)
        if ck == 0:
            valid[:2] = 0.0
        m = {"xh": xhalo, "xl": xl, "wqkv": wqkv, "wproj": proj_w,
             "wfc1": wfc1, "wfc2": fc2_w, "amask": amask, "valid": valid}
        if has_qk_bias:
            m["qkbias"] = np.ascontiguousarray(
                bqkv[:2 * D].reshape(2, 8, 128))
        if has_fc1_bias:
            m["b1"] = bfc1
        in_maps.append(m)

    res = run_bass_kernel_spmd(nc, in_maps, core_ids=list(range(NCORES)))
    y = np.empty((B, N, D), np.float32)
    for c in range(NCORES):
        b, ck = c // 4, c % 4
        y[b, ck * CHUNK:(ck + 1) * CHUNK] = res.results[c]["y"]
    return y


# revision 33
# speedup vs baseline: 1.0232x; 1.0232x over previous
"""Trainium2 Bass kernel for nn_AdvancedTransformerBlock_15006615733156.

Pre-norm transformer block: LN1 -> QKV -> sliding-window causal attention with
ALiBi (window 256) -> proj residual -> LN2 -> FFN (exact GELU) residual.
B=2, N=2048, D=1024, H=16, HD=64.

Sharding: 8 cores = batch(2) x sequence(4 chunks of 512 rows). The 256-wide
sliding window means each core only needs a 256-row halo of x before its
chunk — no collectives. Chunk-0 cores get a zeroed halo plus a `valid` mask
that zeroes halo V' rows (kills both numerator and softmax denominator).

On-chip layout: scores are computed transposed (S_t[kj, qi]) so the
probability tile is directly consumable as matmul lhsT for PV; the softmax
denominator comes from an appended ones-column in V'. All big matmuls run in
float32r (full PE rate, ~1.4e-4 rounding). LN stats use ACT accum_out.
"""
import sys, math, os
sys.path.insert(0, '/opt/trn_rl_repo')
import numpy as np

B, N, D, H, HD, WIN = 2, 2048, 1024, 16, 64, 256
CHUNK, HALO, ROWS = 512, 256, 768
NEG = -1e30
LN_EPS = 1e-5
NCORES = 8

_cache = {}


def _alibi_slopes(n):
    closest = 2 ** math.floor(math.log2(n))
    base = 2.0 ** (-(2.0 ** (-(math.log2(closest) - 3))))
    return np.power(base, np.arange(1, closest + 1)).astype(np.float32)


def _build_amask():
    """Additive pre-softmax bias, [H, 128, 384] bf16.

    Softmax over keys j is invariant to any per-query-column constant, so the
    reference's "+1 inside window" and the -slope*i part of the ALiBi term
    drop out; what remains is slope*(j - i) <= 0 inside the band, -1e30
    outside. Values near each column's max are near zero, so bf16's relative
    rounding cannot disturb the softmax weights meaningfully.
    """
    import ml_dtypes
    sl = _alibi_slopes(H)
    j = np.arange(128)[:, None]
    i = np.arange(384)[None, :]
    band = ((i - j) >= 0) & ((i - j) <= 255)
    out = np.where(band[None], sl[:, None, None] * (j - i)[None], NEG)
    return np.ascontiguousarray(out.astype(ml_dtypes.bfloat16))


def _kb_span(kb):
    qlo = max(0, kb * 128 - 256)
    qhi = min(512, kb * 128 + 128)
    return qlo, qhi, qlo - (kb * 128 - 256)


_KB_OFF = [0]
for _kb in range(6):
    _q0, _q1, _ = _kb_span(_kb)
    _KB_OFF.append(_KB_OFF[-1] + (_q1 - _q0))  # offsets into the 1536-wide S row


def _build_program(has_qk_bias, has_fc1_bias):
    import concourse.bass as bass
    import concourse.tile as tile
    from concourse import bacc, mybir
    from concourse.masks import make_identity

    F32, F32R = mybir.dt.float32, mybir.dt.float32r
    BF16 = mybir.dt.bfloat16
    # 16-bit P/V' runs PV at 1 cyc/row (fp32 pays 4x at N=65) and gets FWL on
    # the weight load; fp16's 10-bit mantissa keeps softmax-prob rounding at
    # ~5e-4 (bf16's 8-bit costs ~4x more accuracy).
    _pv = os.environ.get("K_PV_DT", "f16")
    PV_DT = {"f32": F32, "bf16": BF16, "f16": mybir.dt.float16}[_pv]
    AF = mybir.ActivationFunctionType
    ADD, MULT = mybir.AluOpType.add, mybir.AluOpType.mult

    nc = bacc.Bacc("TRN2", target_bir_lowering=False, debug=False,
                   num_devices=NCORES)

    xh_d = nc.dram_tensor("xh", [HALO, D], F32, kind="ExternalInput").ap()
    xl_d = nc.dram_tensor("xl", [CHUNK, D], F32, kind="ExternalInput").ap()
    wqkv_d = nc.dram_tensor("wqkv", [D, 3 * D], F32R, kind="ExternalInput").ap()
    wproj_d = nc.dram_tensor("wproj", [D, D], F32R, kind="ExternalInput").ap()
    wfc1_d = nc.dram_tensor("wfc1", [D, 4 * D], F32R, kind="ExternalInput").ap()
    wfc2_d = nc.dram_tensor("wfc2", [4 * D, D], F32R, kind="ExternalInput").ap()
    amask_d = nc.dram_tensor("amask", [H, 128, 384], BF16, kind="ExternalInput").ap()
    valid_d = nc.dram_tensor("valid", [6, 128], F32, kind="ExternalInput").ap()
    if has_qk_bias:
        qkb_d = nc.dram_tensor("qkbias", [2, 8, 128], F32, kind="ExternalInput").ap()
    if has_fc1_bias:
        b1_d = nc.dram_tensor("b1", [4 * D], F32, kind="ExternalInput").ap()
    y_d = nc.dram_tensor("y", [CHUNK, D], F32, kind="ExternalOutput").ap()

    def ln_block(tc, x_ap, out_ap, small, dump, eps):
        """LayerNorm (no affine) of [128, 1024]: out = (x - mu) * rstd.

        Stats split across engines in parallel: ACT computes E[x^2] via
        Square+accum while DVE reduces E[x]; var = E[x^2] - mu^2.
        """
        sq = small.tile([128, 1], F32, tag="sq", name="sq")
        nc.scalar.activation(dump[:], x_ap, AF.Square, accum_out=sq[:])
        sums = small.tile([128, 1], F32, tag="sums", name="sums")
        nc.vector.tensor_reduce(sums[:], x_ap, mybir.AxisListType.X,
                                mybir.AluOpType.add)
        negmu = small.tile([128, 1], F32, tag="negmu", name="negmu")
        nc.vector.tensor_scalar_mul(negmu[:], sums[:], -1.0 / D)
        m2 = small.tile([128, 1], F32, tag="m2", name="m2")
        nc.vector.tensor_tensor(m2[:], negmu[:], negmu[:], MULT)
        bvar = small.tile([128, 1], F32, tag="bvar", name="bvar")
        nc.vector.tensor_scalar(bvar[:], m2[:], -1.0, LN_EPS, MULT, ADD)
        st = small.tile([128, 1], F32, tag="st", name="st")
        nc.scalar.activation(st[:], sq[:], AF.Sqrt, bias=bvar[:], scale=1.0 / D)
        rstd = small.tile([128, 1], F32, tag="rstd", name="rstd")
        nc.vector.reciprocal(rstd[:], st[:])
        nmr = small.tile([128, 1], F32, tag="nmr", name="nmr")
        nc.vector.tensor_tensor(nmr[:], negmu[:], rstd[:], MULT)
        nc.vector.tensor_scalar(out_ap, x_ap, rstd[:], nmr[:], MULT, ADD)

    with tile.TileContext(nc) as tc:
        # Pool lifetimes form two LIFO stacks (SBUF left/right). Right holds
        # whole-kernel + B..C + F pools; left holds A..B, C..D, E..F1 chains.
        glob = tc.alloc_tile_pool(name="glob", bufs=1, side="right")
        small = tc.alloc_tile_pool(name="small", bufs=8, side="right")
        de = tc.alloc_tile_pool(name="de", bufs=1, side="right")  # x2 (D..end)

        ident = glob.tile([128, 128], F32, name="ident")
        make_identity(nc, ident[:])
        eps_t = glob.tile([128, 1], F32, name="eps_t")
        nc.vector.memset(eps_t[:], LN_EPS)
        dump = glob.tile([128, D], F32, name="dump")
        x2_sb = de.tile([128, 4, D], F32, name="x2_sb")
        amask_t = glob.tile([128, H, 384], BF16, name="amask_t")

        # ---------------- Phase A: LN1 + h^T ----------------
        xlp = tc.alloc_tile_pool(name="xlp", bufs=1, side="left")  # x local, A..D
        ab = tc.alloc_tile_pool(name="ab", bufs=1, side="left")
        hT = ab.tile([128, 8, ROWS], F32R, name="hT")

        pa = tc.alloc_tile_pool(name="pa", bufs=2, side="left")
        psa = tc.alloc_tile_pool(name="psa", bufs=2, space="PSUM")
        xh_sb = pa.tile([128, 2, D], F32, name="xh_sb", bufs=1)
        # halo first (LN block 0 needs it; HWDGE DMAs are FIFO), split so
        # block 0 lands early
        xh_r = xh_d.rearrange("(q p) d -> p q d", p=128)
        nc.sync.dma_start(xh_sb[:, 0], xh_r[:, 0])
        nc.sync.dma_start(xh_sb[:, 1], xh_r[:, 1])
        xl_sb = xlp.tile([128, 4, D], F32, name="xl_sb")
        xl_r = xl_d.rearrange("(q p) d -> p q d", p=128)
        for q in range(4):
            nc.sync.dma_start(xl_sb[:, q], xl_r[:, q])
        nc.sync.dma_start(amask_t[:], amask_d.rearrange("h p c -> p h c"))
        for blk in range(6):
            x_ap = xh_sb[:, blk] if blk < 2 else xl_sb[:, blk - 2]
            h_pre = pa.tile([128, D], F32, tag="h_pre", name="h_pre")
            ln_block(tc, x_ap, h_pre[:], small, dump, eps_t)
            for kc in range(8):
                pst = psa.tile([128, 128], F32, tag="tr", name="ps_tr")
                nc.tensor.transpose(pst[:], h_pre[:, kc * 128:(kc + 1) * 128],
                                    ident[:])
                if kc % 2 == 0:
                    nc.vector.tensor_copy(hT[:, kc, blk * 128:(blk + 1) * 128], pst[:])
                else:
                    nc.scalar.copy(hT[:, kc, blk * 128:(blk + 1) * 128], pst[:])
        psa.release()
        pa.release()

        # ---------------- Phase B: QKV projections ----------------
        bc = tc.alloc_tile_pool(name="bc", bufs=1, side="right")
        QT = bc.tile([128, 8, CHUNK], F32R, name="QT")     # [hd-pair, pair, qi]
        KT = bc.tile([128, 8, ROWS], F32R, name="KT")
        Vp = bc.tile([128, 6, 16 * 65], PV_DT, name="Vp")    # per-head 65-col groups
        valid_t = bc.tile([128, 6], F32, name="valid_t")
        nc.sync.dma_start(valid_t[:], valid_d.rearrange("k p -> p k"))
        if has_qk_bias:
            qkb_t = bc.tile([128, 2, 8], F32, name="qkb_t")
            nc.sync.dma_start(qkb_t[:], qkb_d.rearrange("t g p -> p t g"))

        wb = tc.alloc_tile_pool(name="wb", bufs=2, side="right")
        psb = tc.alloc_tile_pool(name="psb", bufs=2, space="PSUM")

        # Q: 4 groups of 2 head-pairs (cols 256 each)
        for g in range(4):
            wq = wb.tile([128, 8, 256], F32R, tag="wqk", name="wq")
            nc.sync.dma_start(
                wq[:], wqkv_d[:, g * 256:(g + 1) * 256]
                .rearrange("(ko p) n -> p ko n", p=128))
            for pp in range(2):
                p = g * 2 + pp
                psq = psb.tile([128, CHUNK], F32, tag="q", name="ps_q", bufs=2)
                for ko in range(8):
                    nc.tensor.matmul(psq[:], wq[:, ko, pp * 128:(pp + 1) * 128],
                                     hT[:, ko, HALO:ROWS],
                                     start=(ko == 0), stop=(ko == 7))
                if has_qk_bias:
                    nc.scalar.activation(QT[:, p], psq[:], AF.Identity,
                                         bias=qkb_t[:, 0, p:p + 1])
                else:
                    nc.scalar.copy(QT[:, p], psq[:])

        # K: 4 groups of 2 head-pairs, N=768
        for g in range(4):
            wk = wb.tile([128, 8, 256], F32R, tag="wqk", name="wk")
            nc.sync.dma_start(
                wk[:], wqkv_d[:, D + g * 256:D + (g + 1) * 256]
                .rearrange("(ko p) n -> p ko n", p=128))
            for pp in range(2):
                p = g * 2 + pp
                psk = psb.tile([128, ROWS], F32, tag="k", name="ps_k", bufs=2)
                for n0, n1 in ((0, 512), (512, 768)):
                    for ko in range(8):
                        nc.tensor.matmul(psk[:, n0:n1],
                                         wk[:, ko, pp * 128:(pp + 1) * 128],
                                         hT[:, ko, n0:n1],
                                         start=(ko == 0), stop=(ko == 7))
                if has_qk_bias:
                    nc.scalar.activation(KT[:, p], psk[:], AF.Identity,
                                         bias=qkb_t[:, 1, p:p + 1])
                else:
                    nc.scalar.copy(KT[:, p], psk[:])

        # V: natural layout [rows, feats], assembled into V' with ones col
        wv = wb.tile([128, 8, D], F32R, tag="wv", name="wv", bufs=1)
        nc.sync.dma_start(wv[:],
                          wqkv_d[:, 2 * D:3 * D].rearrange("(ko p) n -> p ko n", p=128))
        for rb in range(6):
            vp_rb = Vp[:, rb].rearrange("p (h c) -> p h c", c=65)
            for nh in range(2):
                psv = psb.tile([128, 512], F32, tag="v", name="ps_v", bufs=2)
                for ko in range(8):
                    nc.tensor.matmul(psv[:],
                                     hT[:, ko, rb * 128:(rb + 1) * 128],
                                     wv[:, ko, nh * 512:(nh + 1) * 512],
                                     start=(ko == 0), stop=(ko == 7))
                # heads nh*8 .. nh*8+8 of this row-block
                nc.vector.tensor_copy(
                    vp_rb[:, nh * 8:(nh + 1) * 8, 0:64],
                    psv[:].rearrange("p (h c) -> p h c", c=64))
            nc.vector.memset(vp_rb[:, :, 64:65], 1.0)
            nc.vector.tensor_scalar_mul(Vp[:, rb], Vp[:, rb], valid_t[:, rb:rb + 1])

        psb.release()
        wb.release()
        ab.release()  # frees hT

        # ---------------- Phase C: attention ----------------
        cd = tc.alloc_tile_pool(name="cd", bufs=1, side="left")
        O_sb = cd.tile([128, 4, D], F32, name="O_sb")

        sp = tc.alloc_tile_pool(name="sp", bufs=2, side="right")
        ddw = tc.alloc_tile_pool(name="ddw", bufs=1, side="left")
        psc = tc.alloc_tile_pool(name="psc", bufs=4, space="PSUM")
        pso = tc.alloc_tile_pool(name="pso", bufs=4, space="PSUM")
        wpj = None
        for hp in range(8):
            if hp == 4:
                # prefetch proj weights while attention still runs
                wpj = ddw.tile([128, 8, D], F32R, name="wpj")
                nc.sync.dma_start(wpj[:],
                                  wproj_d.rearrange("(ko p) n -> p ko n", p=128))
            S_pr = sp.tile([128, 2, 1536], F32, tag="S", name="S_pr")
            P_pr = sp.tile([128, 2, 1536], PV_DT, tag="P", name="P_pr")
            for kb in range(6):
                qlo, qhi, il = _kb_span(kb)
                w = qhi - qlo
                # widen 128-wide spans to 256 so fp32r streams at 1 cyc/row;
                # the extra columns stay in PSUM and are never read.
                mlo = min(qlo, 256) if w >= 256 else (0 if kb == 0 else 256)
                mhi = mlo + max(w, 256)
                voff = qlo - mlo
                # [128, 2, 512]: each head-half starts on a PSUM bank boundary
                pss = psc.tile([128, 2, 512], F32, tag="s", name="ps_s", bufs=2)
                for hh in range(2):
                    pb = hh * 64
                    nc.tensor.matmul(pss[:, hh, :mhi - mlo],
                                     KT[pb:pb + 64, hp, kb * 128:(kb + 1) * 128],
                                     QT[pb:pb + 64, hp, mlo:mhi],
                                     start=True, stop=True)
                nc.vector.tensor_tensor(
                    S_pr[:, :, _KB_OFF[kb]:_KB_OFF[kb] + w],
                    pss[:, :, voff:voff + w],
                    amask_t[:, 2 * hp:2 * hp + 2, il:il + w], ADD)
            nc.scalar.activation(P_pr[:], S_pr[:], AF.Exp)
            for hh in range(2):
                h_i = hp * 2 + hh
                for qb in range(4):
                    po = pso.tile([128, 65], F32, tag="o", name="ps_o")
                    for t in range(3):
                        kb = qb + t
                        qlo, _, _ = _kb_span(kb)
                        pcol = _KB_OFF[kb] + qb * 128 - qlo
                        nc.tensor.matmul(po[:], P_pr[:, hh, pcol:pcol + 128],
                                         Vp[:, kb, h_i * 65:(h_i + 1) * 65],
                                         start=(t == 0), stop=(t == 2))
                    rec = small.tile([128, 1], F32, tag="rec", name="rec")
                    nc.vector.reciprocal(rec[:], po[:, 64:65])
                    if qb % 2 == 0:
                        # balance the divide work across ACT and DVE
                        nc.scalar.activation(
                            O_sb[:, qb, h_i * 64:(h_i + 1) * 64], po[:, 0:64],
                            AF.Copy, scale=rec[:])
                    else:
                        nc.vector.tensor_scalar_mul(
                            O_sb[:, qb, h_i * 64:(h_i + 1) * 64], po[:, 0:64],
                            rec[:])
        pso.release()
        psc.release()
        sp.release()
        bc.release()  # frees QT/KT/Vp

        # ---------------- Phase D: O^T + proj + residual ----------------
        dd = tc.alloc_tile_pool(name="dd", bufs=1, side="left")
        OT = dd.tile([128, 8, CHUNK], F32R, name="OT")
        psd = tc.alloc_tile_pool(name="psd", bufs=2, space="PSUM")
        for qb in range(4):
            for fc in range(8):
                pst = psd.tile([128, 128], F32, tag="tr", name="ps_tr2")
                nc.tensor.transpose(pst[:], O_sb[:, qb, fc * 128:(fc + 1) * 128],
                                    ident[:])
                if fc % 2 == 0:
                    nc.vector.tensor_copy(OT[:, fc, qb * 128:(qb + 1) * 128], pst[:])
                else:
                    nc.scalar.copy(OT[:, fc, qb * 128:(qb + 1) * 128], pst[:])
        for qc in range(4):
            for nh in range(2):
                psp = psd.tile([128, 512], F32, tag="p", name="ps_p")
                for fc in range(8):
                    nc.tensor.matmul(psp[:], OT[:, fc, qc * 128:(qc + 1) * 128],
                                     wpj[:, fc, nh * 512:(nh + 1) * 512],
                                     start=(fc == 0), stop=(fc == 7))
                nc.vector.tensor_tensor(x2_sb[:, qc, nh * 512:(nh + 1) * 512],
                                        psp[:], xl_sb[:, qc, nh * 512:(nh + 1) * 512],
                                        ADD)
        psd.release()
        dd.release()
        ddw.release()
        cd.release()
        xlp.release()

        # ---------------- Phase F pools (right side, before E on left) ------
        ff = tc.alloc_tile_pool(name="ff", bufs=1, side="right")
        ffT = ff.tile([128, 32, CHUNK], F32R, name="ffT")
        y_sb = ff.tile([128, 4, D], F32, name="y_sb")
        if has_fc1_bias:
            b1_t = ff.tile([128, 32], F32, name="b1_t")
            nc.sync.dma_start(b1_t[:], b1_d.rearrange("(fo p) -> p fo", p=128))
        wf = tc.alloc_tile_pool(name="wf", bufs=2, side="right")

        # ---------------- Phase E: LN2 + h2^T ----------------
        ef = tc.alloc_tile_pool(name="ef", bufs=1, side="left")
        h2T = ef.tile([128, 8, CHUNK], F32R, name="h2T")
        pe_ = tc.alloc_tile_pool(name="pe", bufs=2, side="left")
        pse = tc.alloc_tile_pool(name="pse", bufs=2, space="PSUM")
        for qc in range(4):
            h2_pre = pe_.tile([128, D], F32, tag="h2_pre", name="h2_pre")
            ln_block(tc, x2_sb[:, qc], h2_pre[:], small, dump, eps_t)
            for kc in range(8):
                pst = pse.tile([128, 128], F32, tag="tr", name="ps_tr3")
                nc.tensor.transpose(pst[:], h2_pre[:, kc * 128:(kc + 1) * 128],
                                    ident[:])
                if kc % 2 == 0:
                    nc.vector.tensor_copy(h2T[:, kc, qc * 128:(qc + 1) * 128], pst[:])
                else:
                    nc.scalar.copy(h2T[:, kc, qc * 128:(qc + 1) * 128], pst[:])
        pse.release()
        pe_.release()

        # ---------------- Phase F1: fc1 + GELU ----------------
        psf = tc.alloc_tile_pool(name="psf", bufs=2, space="PSUM")
        for g in range(16):
            w1 = wf.tile([128, 8, 256], F32R, tag="w1", name="w1", bufs=3)
            nc.sync.dma_start(
                w1[:], wfc1_d[:, g * 256:(g + 1) * 256]
                .rearrange("(ko p) n -> p ko n", p=128))
            for f4 in range(2):
                ffc = g * 2 + f4
                psq = psf.tile([128, 512], F32, tag="f", name="ps_f")
                for ko in range(8):
                    nc.tensor.matmul(psq[:], w1[:, ko, f4 * 128:(f4 + 1) * 128],
                                     h2T[:, ko, :], start=(ko == 0), stop=(ko == 7))
                if has_fc1_bias:
                    nc.scalar.activation(ffT[:, ffc, :], psq[:], AF.Gelu,
                                         bias=b1_t[:, ffc:ffc + 1])
                else:
                    nc.scalar.activation(ffT[:, ffc, :], psq[:], AF.Gelu)
        psf.release()
        ef.release()

        # ---------------- Phase F2: fc2 + residual + store ----------------
        psy = tc.alloc_tile_pool(name="psy", bufs=1, space="PSUM")
        ys = [psy.tile([128, 512], F32, name=f"ps_y{i}") for i in range(8)]
        for ffc in range(32):
            w2 = wf.tile([128, D], F32R, tag="w2", name="w2", bufs=3)
            nc.sync.dma_start(w2[:], wfc2_d[ffc * 128:(ffc + 1) * 128, :])
            for qc in range(4):
                for nh in range(2):
                    nc.tensor.matmul(ys[qc * 2 + nh][:],
                                     ffT[:, ffc, qc * 128:(qc + 1) * 128],
                                     w2[:, nh * 512:(nh + 1) * 512],
                                     start=(ffc == 0), stop=(ffc == 31))
        y_dr = y_d.rearrange("(q p) d -> p q d", p=128)
        for qc in range(4):
            for nh in range(2):
                nc.vector.tensor_tensor(y_sb[:, qc, nh * 512:(nh + 1) * 512],
                                        ys[qc * 2 + nh][:],
                                        x2_sb[:, qc, nh * 512:(nh + 1) * 512], ADD)
            nc.sync.dma_start(y_dr[:, qc], y_sb[:, qc])
        psy.release()
        wf.release()
        ff.release()
        de.release()
        small.release()
        glob.release()

    nc.compile()
    return nc


def kernel(x, qkv_w, qkv_b, proj_w, proj_b, ln1_g, ln1_b, ln2_g, ln2_b,
           fc1_w, fc1_b, fc2_w, fc2_b):
    from concourse.bass_utils import run_bass_kernel_spmd

    x = np.ascontiguousarray(np.asarray(x, dtype=np.float32))
    f32 = lambda a: np.asarray(a, dtype=np.float32)
    qkv_w, qkv_b = f32(qkv_w), f32(qkv_b)
    proj_w, proj_b = f32(proj_w), f32(proj_b)
    fc1_w, fc1_b = f32(fc1_w), f32(fc1_b)
    fc2_w, fc2_b = f32(fc2_w), f32(fc2_b)
    ln1_g, ln1_b = f32(ln1_g), f32(ln1_b)
    ln2_g, ln2_b = f32(ln2_g), f32(ln2_b)

    # Host-side folding: LN affine into the following weight/bias; HD^-0.5 into Wk.
    scale = HD ** -0.5
    wqkv = ln1_g[:, None] * qkv_w
    bqkv = qkv_b + ln1_b @ qkv_w
    wqkv = np.ascontiguousarray(wqkv)
    wqkv[:, D:2 * D] *= scale
    bqkv = bqkv.copy()
    bqkv[D:2 * D] *= scale
    wfc1 = np.ascontiguousarray(ln2_g[:, None] * fc1_w)
    bfc1 = fc1_b + ln2_b @ fc1_w

    if np.any(bqkv[2 * D:]) or np.any(proj_b) or np.any(fc2_b):
        raise NotImplementedError("nonzero v/proj/fc2 bias not supported")

    has_qk_bias = bool(np.any(bqkv[:2 * D]))
    has_fc1_bias = bool(np.any(bfc1))
    key = (has_qk_bias, has_fc1_bias)
    if key not in _cache:
        _cache[key] = _build_program(*key)
    nc = _cache[key]

    amask = _build_amask()
    in_maps = []
    for c in range(NCORES):
        b, ck = c // 4, c % 4
        g0 = ck * CHUNK
        xl = np.ascontiguousarray(x[b, g0:g0 + CHUNK])
        if ck > 0:
            xhalo = np.ascontiguousarray(x[b, g0 - HALO:g0])
        else:
            xhalo = np.zeros((HALO, D), np.float32)
        valid = np.ones((6, 128), np.float32)
        if ck == 0:
            valid[:2] = 0.0
        m = {"xh": xhalo, "xl": xl, "wqkv": wqkv, "wproj": proj_w,
             "wfc1": wfc1, "wfc2": fc2_w, "amask": amask, "valid": valid}
        if has_qk_bias:
            m["qkbias"] = np.ascontiguousarray(
                bqkv[:2 * D].reshape(2, 8, 128))
        if has_fc1_bias:
            m["b1"] = bfc1
        in_maps.append(m)

    res = run_bass_kernel_spmd(nc, in_maps, core_ids=list(range(NCORES)))
    y = np.empty((B, N, D), np.float32)
    for c in range(NCORES):
        b, ck = c // 4, c % 4
        y[b, ck * CHUNK:(ck + 1) * CHUNK] = res.results[c]["y"]
    return y
